# revision 10
# baseline (speedup 1.0000x reference)
"""GRU (MDGRU) Trainium2 kernel.

Problem: X [64, 2048, 256] fp32, kernel [256, 768], recurrent_kernel [256, 768],
bias [768] -> final GRU state a [64, 256].

Strategy (8 NeuronCores, batch-sharded 8 rows/core):
  Phase 1: projT = X @ kernel, computed in fp16 via xbar DMA-transpose of X
           (d onto partitions), stored per-(b, t) contiguous in DRAM as fp16.
  Phase 2: sequential recurrence in "transposed layout": state aT is
           [u (2x128 partitions-chunks), b=8], recurrent weights stationary
           on the PE as 12 [128,128] fp16 tiles, gate pre-activations
           accumulated in a single PSUM bank on top of (proj + bias) preload,
           sigmoid/tanh on ACT, blend on DVE, everything fp16 except PSUM.
"""

import os
from contextlib import ExitStack

import numpy as np

import concourse.bass as bass
import concourse.mybir as mybir
import concourse.tile as tile
from concourse.bass_utils import run_bass_kernel_spmd

dt = mybir.dt
AF = mybir.ActivationFunctionType
ALU = mybir.AluOpType

B_FULL, T_FULL, D, U = 64, 2048, 256, 256
N_CORES = 8
B = B_FULL // N_CORES  # 8 batch rows per core


def _split_multiwaits(nc):
    """This walrus build allows only one sync-wait per instruction; hoist
    extras onto NoOps inserted just before the offending instruction."""
    n = 0
    for f in nc.m.functions:
        for bb in f.blocks:
            out = []
            for inst in bb.instructions:
                si = inst.sync_info
                if si is not None and si.on_wait is not None and len(si.on_wait) > 1:
                    waits = list(si.on_wait)
                    for w in waits[:-1]:
                        n += 1
                        nop = mybir.InstNoOp(name=f"I-msplit-{n}", ins=[], outs=[])
                        nop.engine = inst.engine
                        nop.sync_info = mybir.SyncInfo(on_wait=[w], on_update=[])
                        out.append(nop)
                    si.on_wait = [waits[-1]]
                out.append(inst)
            bb.instructions = out
    return n


def build_gru(T):
    TC = min(128, T)   # phase-1 t-chunk
    TB = min(128, T)   # phase-2 t-block (xbar source partition dim must be <=128)
    assert T % TB == 0 and T % TC == 0

    nc = bass.Bass("TRN2", target_bir_lowering=False, debug=False, num_devices=1)
    X = nc.dram_tensor("x", [B, T, D], dt.float32, kind="ExternalInput").ap()
    wk = nc.dram_tensor("wk", [2, 128, 768], dt.float16, kind="ExternalInput").ap()
    wr = nc.dram_tensor("wr", [2, 6, 128, 128], dt.float16, kind="ExternalInput").ap()
    b48 = nc.dram_tensor("b48", [128, 48], dt.float16, kind="ExternalInput").ap()
    ident = nc.dram_tensor("ident", [128, 128], dt.float16, kind="ExternalInput").ap()
    out = nc.dram_tensor("out", [B, U], dt.float32, kind="ExternalOutput").ap()

    with tile.TileContext(nc) as tc, ExitStack() as ctx:
        dram = ctx.enter_context(tc.tile_pool(name="dram", bufs=1, space="DRAM"))
        consts = ctx.enter_context(tc.tile_pool(name="consts", bufs=1))
        p1 = ctx.enter_context(tc.tile_pool(name="p1", bufs=3))
        p1psum = ctx.enter_context(tc.tile_pool(name="p1psum", bufs=2, space="PSUM"))
        projp = ctx.enter_context(tc.tile_pool(name="projp", bufs=2))
        psum2 = ctx.enter_context(tc.tile_pool(name="psum2", bufs=4, space="PSUM"))
        statep = ctx.enter_context(tc.tile_pool(name="statep", bufs=1))
        small = ctx.enter_context(tc.tile_pool(name="small", bufs=2))

        x16 = dram.tile([B, T, D], dt.float16, name="x16")
        projT = dram.tile([B, T, 768], dt.float16, name="projT")

        # ---- constants into SBUF ----
        wk_sb = consts.tile([128, 2 * 768], dt.float16, name="wk_sb")
        for k in range(2):
            nc.sync.dma_start(wk_sb[:, k * 768:(k + 1) * 768], wk[k])
        wr_sb = consts.tile([128, 12 * 128], dt.float16, name="wr_sb")
        for k in range(2):
            for m in range(6):
                i = k * 6 + m
                nc.sync.dma_start(wr_sb[:, i * 128:(i + 1) * 128], wr[k, m])
        b48_sb = consts.tile([128, 48], dt.float16, name="b48_sb")
        nc.sync.dma_start(b48_sb[:], b48)
        id_sb = consts.tile([128, 128], dt.float16, name="id_sb")
        nc.sync.dma_start(id_sb[:], ident)

        # ---- phase 1: projT[b, t, :] = (X[b] @ kernel) in fp16 ----
        for b in range(B):
            nc.gpsimd.dma_start(x16[b], X[b])  # fp32 -> fp16 cast DMA

        def wr_tile(k, m):
            i = k * 6 + m
            return wr_sb[:, i * 128:(i + 1) * 128]

        for tci in range(T // TC):
            for b in range(B):
                xt = p1.tile([128, 2 * TC], dt.float16, name="xt", tag="xt")
                for k in range(2):
                    nc.sync.dma_start(
                        xt[:, k * TC:(k + 1) * TC],
                        x16[b, tci * TC:(tci + 1) * TC, k * 128:(k + 1) * 128],
                        transpose=True,
                    )
                ps = p1psum.tile([128, 768], dt.float32, name="ps", tag="ps")
                for k in range(2):
                    lhsT = xt[:, k * TC:(k + 1) * TC]
                    nc.tensor.matmul(ps[0:TC, 0:512], lhsT,
                                     wk_sb[:, k * 768:k * 768 + 512],
                                     start=(k == 0), stop=(k == 1))
                    nc.tensor.matmul(ps[0:TC, 512:768], lhsT,
                                     wk_sb[:, k * 768 + 512:(k + 1) * 768],
                                     start=(k == 0), stop=(k == 1))
                ev = p1.tile([128, 768], dt.float16, name="ev", tag="ev")
                nc.vector.tensor_copy(ev[0:TC, :], ps[0:TC, :])
                nc.sync.dma_start(projT[b, tci * TC:(tci + 1) * TC, :], ev[0:TC, :])

        # ---- phase 2: recurrence ----
        aT = [statep.tile([128, 16], dt.float16, name=f"aT{i}") for i in range(2)]
        nc.vector.memset(aT[0][:], 0.0)

        pt_view = None
        for blk in range(T // TB):
            pt = projp.tile([128, 48 * TB], dt.float16, name="pt", tag="pt")
            ptv = pt.rearrange("p (c t) -> p c t", t=TB)
            for m in range(6):
                for b in range(B):
                    c = m * 8 + b
                    nc.sync.dma_start(
                        pt[:, c * TB:(c + 1) * TB],
                        projT[b, blk * TB:(blk + 1) * TB, m * 128:(m + 1) * 128],
                        transpose=True,
                    )
            for ti in range(TB):
                t = blk * TB + ti
                cur = aT[t % 2]
                nxt = aT[(t + 1) % 2]
                ps2 = psum2.tile([128, 48], dt.float32, name="ps2", tag="ps2",
                                 bufs=3)
                nc.vector.tensor_tensor(ps2[:], ptv[:, :, ti],
                                        b48_sb[:], ALU.add)
                for m in range(4):
                    for k in range(2):
                        nc.tensor.matmul(ps2[:, m * 8:(m + 1) * 8], wr_tile(k, m),
                                         cur[:, k * 8:(k + 1) * 8],
                                         start=False, stop=(k == 1),
                                         skip_group_check=True)
                zr = small.tile([128, 32], dt.float16, name="zr", tag="zr")
                nc.scalar.activation(zr[:], ps2[:, 0:32], AF.Sigmoid)
                ra = small.tile([128, 16], dt.float16, name="ra", tag="ra")
                nc.vector.tensor_mul(ra[:], zr[:, 16:32], cur[:])
                for m in range(4, 6):
                    for k in range(2):
                        nc.tensor.matmul(ps2[:, m * 8:(m + 1) * 8], wr_tile(k, m),
                                         ra[:, k * 8:(k + 1) * 8],
                                         start=False, stop=(k == 1),
                                         skip_group_check=True)
                hs = small.tile([128, 16], dt.float16, name="hs", tag="hs")
                nc.scalar.activation(hs[:], ps2[:, 32:48], AF.Tanh)
                dd = small.tile([128, 16], dt.float16, name="dd", tag="dd")
                nc.vector.tensor_sub(dd[:], cur[:], hs[:])
                dz = small.tile([128, 16], dt.float16, name="dz", tag="dz")
                nc.vector.tensor_mul(dz[:], dd[:], zr[:, 0:16])
                nc.vector.tensor_add(nxt[:], dz[:], hs[:])

        # ---- output: transpose aT [u, (k,b)] -> [(k,b), u] and store ----
        a_fin = aT[T % 2]
        pso = psum2.tile([16, 128], dt.float16, name="pso", bufs=1)
        nc.tensor.transpose(pso[:], a_fin[:], id_sb[:])
        ot = consts.tile([16, 128], dt.float32, name="ot")
        nc.vector.tensor_copy(ot[:], pso[:])
        for k in range(2):
            nc.sync.dma_start(out[:, k * 128:(k + 1) * 128], ot[k * 8:(k + 1) * 8, :])

    return nc


def _prep_weights(kernel, recurrent_kernel, bias):
    wk = np.ascontiguousarray(kernel.reshape(2, 128, 768)).astype(np.float16)
    wr = np.ascontiguousarray(
        recurrent_kernel.reshape(2, 128, 6, 128).transpose(0, 2, 1, 3)
    ).astype(np.float16)
    b48 = np.ascontiguousarray(
        np.repeat(bias.reshape(6, 128).T[:, :, None], 8, axis=2).reshape(128, 48)
    ).astype(np.float16)
    ident = np.eye(128, dtype=np.float16)
    return wk, wr, b48, ident


_BUILD_CACHE = {}


def run_gru(X, kernel, recurrent_kernel, bias, T=None, trace=False):
    T = T if T is not None else X.shape[1]
    if T not in _BUILD_CACHE:
        _BUILD_CACHE[T] = build_gru(T)
    nc = _BUILD_CACHE[T]
    if not getattr(nc, "_mw_split", False):
        _split_multiwaits(nc)
        nc._mw_split = True
    wk, wr, b48, ident = _prep_weights(
        np.asarray(kernel, dtype=np.float32),
        np.asarray(recurrent_kernel, dtype=np.float32),
        np.asarray(bias, dtype=np.float32),
    )
    X = np.asarray(X, dtype=np.float32)
    n_b = X.shape[0]
    assert n_b == B * N_CORES
    in_maps = []
    for c in range(N_CORES):
        in_maps.append({
            "x": np.ascontiguousarray(X[c * B:(c + 1) * B]),
            "wk": wk, "wr": wr, "b48": b48, "ident": ident,
        })
    res = run_bass_kernel_spmd(nc, in_maps, core_ids=list(range(N_CORES)),
                               trace=trace)
    outp = np.concatenate([res.results[c]["out"] for c in range(N_CORES)], axis=0)
    return outp, res


def kernel(X, kernel, recurrent_kernel, bias):
    outp, _ = run_gru(X, kernel, recurrent_kernel, bias)
    return outp.astype(np.float32)


if __name__ == "__main__":
    # small self-check against a numpy GRU at T=256
    def ref_np(X, Wk, R, b):
        Uz, Ur, Uh = R[:, :U], R[:, U:2 * U], R[:, 2 * U:]
        proj = X.astype(np.float64) @ Wk.astype(np.float64) + b.astype(np.float64)
        a = np.zeros((X.shape[0], U))
        sig = lambda v: 1.0 / (1.0 + np.exp(-v))
        for t in range(X.shape[1]):
            xz, xr, xh = proj[:, t, :U], proj[:, t, U:2 * U], proj[:, t, 2 * U:]
            z = sig(a @ Uz + xz)
            r = sig(a @ Ur + xr)
            h = np.tanh((r * a) @ Uh + xh)
            a = z * a + (1.0 - z) * h
        return a

    T = int(os.environ.get("GRU_T", "256"))
    rng = np.random.default_rng(0)
    X = rng.standard_normal((B_FULL, T, D), dtype=np.float32)
    Wk = (rng.standard_normal((D, 3 * U), dtype=np.float32) / np.sqrt(D))
    R = (rng.standard_normal((U, 3 * U), dtype=np.float32) / np.sqrt(U))
    b = rng.standard_normal((3 * U,), dtype=np.float32) * 0.01
    import time
    t0 = time.time()
    got, _ = run_gru(X, Wk, R, b, T=T)
    print(f"device roundtrip: {time.time() - t0:.1f}s")
    want = ref_np(X, Wk, R, b)
    err = np.abs(got - want)
    rel = np.max(err) / np.max(np.abs(want))
    print(f"T={T} max abs err {np.max(err):.3e}  rel(maxabs) {rel:.3e}")
